# revision 52
# baseline (speedup 1.0000x reference)
"""BasicLS on 8 trn2 cores — fp16 stats + scaled solve, engine-balanced.

Pipeline per 4096-batch tile:
  1. DMA x tile [128, (c,m,d)] fp32.
  2. ACT cast+swizzle -> xh [128, (d, q, g, m)] fp16   (c = 4q+g)
  3. PE transposes of the 32 [128,(g,m)] blocks -> PSUM; dpair0 copied
     PSUM->SBUF on DVE, dpair1 via DMA -> Fall [128=(g,m), (d, q, p)] fp16.
  4. fp16 products: 6 cross on DVE, 1 square on DVE, 2 squares on ACT.
  5. PE matmuls with a sliding ones-pattern weight reduce over m into
     PSUM stats [52=(4s+g), 512]; fp32 accumulation.
  6. ACT copy PSUM->SBUF with 1/32 scale -> fp16 stats; PE transposes
     stat chunks back (fp16) -> ST2 [128=p, t, q, 52=(4s+g)] fp16.
  7. Solve on scaled stats (pivot 1): Schur-eliminate, symmetric 3x3
     adjugate; fp16 temporaries, fp32 det/recip chain. Half 0 runs
     entirely on GPSIMD overlapping the tiles 4..7 stats; half 1 is the
     only tail. Output DMA'd per half.
"""

import itertools
import os as _os

import numpy as np

import concourse.bacc as bacc
import concourse.tile as tile
from concourse import mybir
from concourse.bass_utils import run_bass_kernel_spmd
from concourse.masks import make_identity

F32 = mybir.dt.float32
F16 = mybir.dt.float16

B, M, D = 262144, 32, 4
NCORES = 8
BC = B // NCORES          # 32768
NT = 8
TB = BC // NT             # 4096
CPT = TB // 128           # 32 (c = 4q + g, q:8, g:4)
NQ, NG = 8, 4
W = NT * CPT              # 256

# stat order: 0..3 = T0..T3; 4 S01, 5 S02, 6 S03, 7 S11, 8 S12, 9 S13,
# 10 S22, 11 S23, 12 S33
CROSS = [(4, 0, 1), (5, 0, 2), (6, 0, 3), (8, 1, 2), (9, 1, 3), (11, 2, 3)]
SQ = [(7, 1), (10, 2), (12, 3)]
NS = 13


def _emit(nc, tc, xd, yd):
    V, G, A = nc.vector, nc.gpsimd, nc.scalar

    x_all = xd.ap().rearrange("(t p c) m d -> t p c m d", t=NT, p=128)
    y_all = yd.ap().rearrange("(t p c) d -> p t c d", t=NT, p=128)

    with (
        tc.tile_pool(name="const", bufs=1) as cpool,
        tc.tile_pool(name="xin", bufs=3) as xpool,
        tc.tile_pool(name="xh", bufs=3) as xhpool,
        tc.tile_pool(name="fall", bufs=3) as fpool,
        tc.tile_pool(name="pr", bufs=12) as prpool,
        tc.tile_pool(name="sst", bufs=4) as sspool,
        tc.tile_pool(name="stat", bufs=1) as spool,
        tc.tile_pool(name="solve", bufs=1) as lpool,
        tc.tile_pool(name="pp", bufs=6) as pppool,
        tc.tile_pool(name="acc", bufs=4) as apool,
        tc.tile_pool(name="pst", bufs=2, space="PSUM") as ptpool,
        tc.tile_pool(name="psp", bufs=3, space="PSUM") as sppool,
        tc.tile_pool(name="ps2", bufs=1, space="PSUM") as p2pool,
    ):
        ident16 = cpool.tile([128, 128], F16, name="ident16")
        make_identity(nc, ident16)
        # master ones-pattern weight: MW[32g+m, 48+g] = 1.
        # For stat s, lhsT = MW[:, 48-4s : 100-4s] places the group-g m-sum
        # of the rhs at output partition 4s+g.
        MW = cpool.tile([128, 100], F16, name="MW")
        V.memset(MW, 0.0)
        for g in range(NG):
            V.memset(MW[32 * g:32 * (g + 1), 48 + g:49 + g], 1.0)

        _skip_stats = _os.environ.get("KB_SKIP_STATS") == "1"
        _skip_solve = _os.environ.get("KB_SKIP_SOLVE") == "1"
        HT = NT // 2  # legacy name; halves are asymmetric below
        H0N, H1N = 4, 4  # tiles per solve half: big half overlaps on GPSIMD

        # per-batch stats (scaled by 1/32, fp16), split by t-half so the
        # first solve half's dependencies close after tile 3
        ST2h = [
            spool.tile([128, n, NQ, 52], F16, name=f"ST2_{hh}",
                       tag=f"ST2_{hh}")
            for hh, n in enumerate((H0N, H1N))
        ]
        if _skip_stats:
            V.memset(ST2h[0], 1.0)
            V.memset(ST2h[1], 1.0)

        OUT = lpool.tile([128, NT, CPT, D], F32, tag="OUT", name="OUT")
        OUT5 = OUT.rearrange("p t (q g) d -> p t q g d", q=NQ)

        def emit_solve(hh, tl=0, th=None, engines=None):
            """Generator: yields at chunk boundaries so emission can be
            interleaved with tile fronts (keeps engine streams from
            head-of-line blocking on solve deps)."""
            if th is None:
                th = H0N if hh == 0 else H1N

            def stat(s):
                return ST2h[hh][:, tl:th, :, 4 * s:4 * s + 4]

            a, b, c_, d = stat(7), stat(8), stat(9), stat(1)
            e, f_, g_ = stat(10), stat(11), stat(2)
            h, i_ = stat(12), stat(3)
            r0, r1, r2, r3 = stat(4), stat(5), stat(6), stat(0)

            # half 0 overlaps the tiles 4..7 stats work: pure GPSIMD so no
            # tile-engine stream is ever head-of-line blocked by solve deps.
            # half 1 is the tail: pure DVE — a single in-order stream has no
            # cross-engine semaphore hops on the critical path.
            sched = itertools.cycle(
                engines or ([G] if hh == 0 else [V, V, V, G]))
            SH = [128, th - tl, NQ, 4]

            def tmp(name, dt=F16, pool=None, tag=None):
                pool = pool or lpool
                name = f"{name}_h{hh}_{tl}"
                return pool.tile(SH, dt, tag=tag or name, name=name)

            qb = (0 if hh == 0 else H0N) + tl  # absolute tile base

            def emul(u, v, name, dt=F16, pool=None, tag=None):
                t_ = tmp(name, dt, pool, tag)
                next(sched).tensor_mul(out=t_, in0=u, in1=v)
                return t_

            def esub(u, v, name, dt=F16, pool=None, tag=None):
                t_ = tmp(name, dt, pool, tag)
                next(sched).tensor_sub(out=t_, in0=u, in1=v)
                return t_

            def eadd(u, v, name, dt=F16, pool=None, tag=None):
                t_ = tmp(name, dt, pool, tag)
                next(sched).tensor_add(out=t_, in0=u, in1=v)
                return t_

            pptag = f"pp{hh}_{tl}"

            def esq(u, name):  # u*u; ACT (idle in the tail) for half 1
                t_ = tmp(name)
                next(sched).tensor_mul(out=t_, in0=u, in1=u)
                return t_

            def m2(u, v, w, x_, name):  # u*v - w*x
                p1 = emul(u, v, name + "p1", F16, pppool, pptag)
                p2 = emul(w, x_, name + "p2", F16, pppool, pptag)
                return esub(p1, p2, name)

            # ---- Schur elimination of column 4 (pivot 1 after 1/32 scale) --
            dd = esq(d, "dd")
            gg = esq(g_, "gg")
            ii = esq(i_, "ii")
            dg = emul(d, g_, "dg", F16, pppool, pptag)
            di = emul(d, i_, "di", F16, pppool, pptag)
            gi = emul(g_, i_, "gi", F16, pppool, pptag)
            ap_ = esub(a, dd, "ap")
            bp = esub(b, dg, "bp")
            cp = esub(c_, di, "cp")
            ep = esub(e, gg, "ep")
            fp = esub(f_, gi, "fp")
            hp = esub(h, ii, "hp")
            p1_ = emul(r3, d, "c1p", F16, pppool, pptag)
            c1 = esub(r0, p1_, "c1")
            p2_ = emul(r3, g_, "c2p", F16, pppool, pptag)
            c2 = esub(r1, p2_, "c2")
            p3_ = emul(r3, i_, "c3p", F16, pppool, pptag)
            c3 = esub(r2, p3_, "c3")
            yield

            # ---- symmetric 3x3 adjugate solve ----
            fp2 = esq(fp, "fp2")
            cp2 = esq(cp, "cp2")
            bp2 = esq(bp, "bp2")
            eh = emul(ep, hp, "eh", F16, pppool, pptag)
            A11 = esub(eh, fp2, "A11")
            ah = emul(ap_, hp, "ah", F16, pppool, pptag)
            A22 = esub(ah, cp2, "A22")
            ae_ = emul(ap_, ep, "ae3", F16, pppool, pptag)
            A33 = esub(ae_, bp2, "A33")
            A12 = m2(cp, fp, bp, hp, "A12")
            A13 = m2(bp, fp, cp, ep, "A13")
            A23 = m2(cp, bp, ap_, fp, "A23")
            yield

            def dot3(u1, v1, u2, v2, u3, v3, name, mdt=F16):
                q1 = emul(u1, v1, name + "q1", mdt, pppool, pptag)
                q2 = emul(u2, v2, name + "q2", mdt, pppool, pptag)
                s_ = eadd(q1, q2, name + "s", mdt, apool, f"acc{hh}_{tl}")
                q3 = emul(u3, v3, name + "q3", mdt, pppool, pptag)
                return eadd(s_, q3, name, F32)

            det3 = dot3(ap_, A11, bp, A12, cp, A13, "det3")
            n1 = dot3(A11, c1, A12, c2, A13, c3, "n1")
            n2 = dot3(A12, c1, A22, c2, A23, c3, "n2")
            n3 = dot3(A13, c1, A23, c2, A33, c3, "n3")
            yield

            # z4 = (c4*det3 - (d n1 + g n2 + i n3)) * rdet
            dn = dot3(d, n1, g_, n2, i_, n3, "dn", F32)
            c4det = emul(r3, det3, "c4det", F32, pppool, pptag)
            diff = esub(c4det, dn, "diff", F32)
            yield

            # DVE recip emitted post-tile-7 for half 0: it waits on GPSIMD's
            # det3, and emitting it earlier head-of-line blocks tile work
            # queued behind it in the DVE stream.
            rdet = tmp("rdet", F32)
            scratch = tmp("rscratch", F32)
            V.reciprocal_approx_accurate(
                out=rdet.rearrange("p t q g -> p (t q g)"),
                in_=det3.rearrange("p t q g -> p (t q g)"),
                scratch=scratch.rearrange("p t q g -> p (t q g)"),
            )
            nrdet = tmp("nrdet", F32)
            V.tensor_scalar_mul(out=nrdet, in0=rdet, scalar1=-1.0)

            for comp, (dv, rv) in enumerate(
                [(n1, nrdet), (n2, nrdet), (n3, nrdet), (diff, rdet)]
            ):
                next(sched).tensor_mul(
                    out=OUT5[:, qb:qb + (th - tl), :, :, comp],
                    in0=dv, in1=rv,
                )

        def emit_tile(t):
            # Tiles 0..3: GPSIMD helps with tile work (it is otherwise idle).
            # Tiles 4..7: GPSIMD runs solve half 0, so its share moves to V/A.
            use_g = t < NT // 2
            xh = xhpool.tile([128, D, NQ, NG * M], F16, tag="xh", name="xh")
            if t == 0:
                # Split tile 0's DMA + cast by q-half so the first PE
                # transposes start after a half-tile DMA, not the full one.
                for h in range(4):
                    xth = cpool.tile([128, CPT // 4, M, D], F32,
                                     tag=f"xt0_{h}", name=f"xt0_{h}")
                    nc.sync.dma_start(
                        out=xth, in_=x_all[0][:, h * 8:(h + 1) * 8])
                    xswh = xth.rearrange("p (q g) m d -> p d q (g m)", q=2)
                    A.copy(out=xh[:, 0:2, 2 * h:2 * h + 2], in_=xswh[:, 0:2])
                    G.tensor_copy(
                        out=xh[:, 2:4, 2 * h:2 * h + 2], in_=xswh[:, 2:4])
            else:
                xt = xpool.tile([128, CPT, M, D], F32, tag="xt", name="xt")
                # 4 sub-DMAs: short DMA_ENGINES holds so the xbar transposes
                # below never queue behind a monolithic 5.8us transfer.
                for h4 in range(4):
                    nc.sync.dma_start(out=xt[:, 8 * h4:8 * h4 + 8],
                                      in_=x_all[t][:, 8 * h4:8 * h4 + 8])
                # cast + swizzle (c,m,d) -> (d, q, g, m) in one ACT pass
                xsw = xt.rearrange("p (q g) m d -> p d q (g m)", q=NQ)
                A.copy(out=xh[:, 0:2], in_=xsw[:, 0:2])
                if use_g:
                    G.tensor_copy(out=xh[:, 2:4], in_=xsw[:, 2:4])
                else:
                    A.copy(out=xh[:, 2:4], in_=xsw[:, 2:4])
            # d0/d1: PE transposes ([128,(g,m)] -> [(g,m),128]) + DVE copy.
            # d2/d3: xbar DMA transpose straight into SBUF (the DMA engines
            # are ~50% idle; this sheds PE rows and the PSUM copy-back).
            fall = fpool.tile([128, D, NQ, 128], F16, tag="fall", name="fall")
            pt = ptpool.tile([128, 2, NQ, 128], F16, tag="pt", name="pt")
            for dd in range(2):
                for q in range(NQ):
                    nc.tensor.transpose(
                        pt[:, dd, q, :], xh[:, dd, q, :], ident16
                    )
            V.tensor_copy(out=fall[:, 0:2, :, :], in_=pt)
            A.dma_start_transpose(out=fall[:, 2], in_=xh[:, 2])
            A.dma_start_transpose(out=fall[:, 3], in_=xh[:, 3])
            # fp16 products (feature-major, unit stride)
            prods = {}
            for ci, (s, j, k) in enumerate(CROSS):
                pr = prpool.tile([128, NQ, 128], F16, tag="pr", name="pr")
                V.tensor_mul(out=pr, in0=fall[:, j], in1=fall[:, k])
                prods[s] = pr
            for sqi, (s, j) in enumerate(SQ):
                pr = prpool.tile([128, NQ, 128], F16, tag="pr", name="pr")
                if sqi == 1:
                    A.square(out=pr, in_=fall[:, j])
                else:
                    eng = V if (sqi == 0 or not use_g) else G
                    eng.tensor_mul(out=pr, in0=fall[:, j], in1=fall[:, j])
                prods[s] = pr
            # PE reduce over m: 13 accumulating matmuls per 512-col window
            SPT[t] = []
            SORDER = [0, 1, 4, 7, 2, 5, 8, 10, 3, 6, 9, 11, 12]
            for w in range(2):
                spt = sppool.tile([52, 512], F32, tag="spt", name="spt")
                for si, s in enumerate(SORDER):
                    if s < 4:
                        rhs = fall[:, s, 4 * w:4 * w + 4, :]
                    else:
                        rhs = prods[s][:, 4 * w:4 * w + 4, :]
                    nc.tensor.matmul(
                        spt,
                        MW[:, 48 - 4 * s:100 - 4 * s],
                        rhs,
                        start=(si == 0),
                        stop=(si == NS - 1),
                    )
                SPT[t].append(spt)

        SPT, SST = {}, {}

        def emit_epi_a(t):
            # scale 1/32 (pivot -> 1) + cast fp16 during PSUM->SBUF.
            # Emitted AFTER tile t+1's front so ACT's in-order stream never
            # parks tile t+1's casts behind this PE-stats-dependent copy.
            SST[t] = []
            for w in range(2):
                sst = sspool.tile([52, 512], F16, tag="sst", name="sst")
                A.mul(out=sst, in_=SPT[t][w], mul=1.0 / 32.0)
                SST[t].append(sst)

        def emit_epi_b(t):
            # transpose stats back: chunks [52, 128] -> [128, 52] fp16
            pt2 = p2pool.tile([128, 2, 4, 52], F16, tag="pt2", name="pt2")
            for w in range(2):
                for cidx in range(4):
                    nc.tensor.transpose(
                        pt2[:, w, cidx, :],
                        SST[t][w][:, cidx * 128:(cidx + 1) * 128],
                        ident16[0:52, 0:52],
                    )
            for w in range(2):
                hh_ = 0 if t < H0N else 1
                dst = ST2h[hh_][:, t - hh_ * H0N, 4 * w:4 * w + 4, :]
                if w == 0:
                    V.tensor_copy(out=dst, in_=pt2[:, 0])
                else:
                    A.copy(out=dst, in_=pt2[:, 1])
            del SPT[t], SST[t]

        def emit_all_tiles(pump_cb=None):
            """Software-pipelined emission: front(t+1) goes out before tile
            t's stats epilogue, so no engine stream serializes a next-tile
            producer behind a this-tile consumer. Solve-half-0 chunks are
            interleaved once its stats (tiles 0..3) are complete."""
            if _skip_stats:
                return
            for t in range(NT):
                emit_tile(t)
                if t >= 1:
                    emit_epi_a(t - 1)
                    emit_epi_b(t - 1)
                if pump_cb is not None:
                    pump_cb(t)
            emit_epi_a(NT - 1)
            emit_epi_b(NT - 1)
            if pump_cb is not None:
                pump_cb(NT)

        if _skip_solve:
            emit_all_tiles()
            OUTs = lpool.tile([128, NT, CPT, D], F32, tag="OUT", name="OUTs")
            V.tensor_copy(out=OUTs, in_=ST2h[0].rearrange(
                "p t q s -> p (t q s)")[:, 0:NT * CPT * D].rearrange(
                "p (t c d) -> p t c d", t=NT, c=CPT))
            nc.sync.dma_start(out=y_all, in_=OUTs)
            return

        # Four quarter-solves: Q_k covers tiles (2k, 2k+1) and can start
        # as soon as its stats land (epi_b(2k+1)). Q0/Q1 run pure-GPSIMD
        # during the PE phase; only Q3 (plus Q2 spill) is tail. y DMAs ride
        # GPSIMD's SWDGE (no tile work queued behind them).
        YSL = [(0, H0N), (H0N, NT)]
        QS = [emit_solve(0, engines=[G]), None]
        START = [H0N, 99]

        def pump(t):
            for k, gen in enumerate(QS):
                if gen is not None and t >= START[k]:
                    if next(gen, "done") == "done":
                        QS[k] = None
                        lo, hi = YSL[k]
                        G.dma_start(out=y_all[:, lo:hi], in_=OUT[:, lo:hi])

        emit_all_tiles(pump_cb=pump)
        for k, gen in enumerate([QS[0], emit_solve(1, engines=[V, V, V, G])]):
            if gen is not None:
                for _ in gen:
                    pass
                lo, hi = YSL[k]
                G.dma_start(out=y_all[:, lo:hi], in_=OUT[:, lo:hi])


_NC_CACHE = {}


def _get_nc():
    if "nc" not in _NC_CACHE:
        nc = bacc.Bacc("TRN2", target_bir_lowering=False, debug=False,
                       num_devices=NCORES)
        xd = nc.dram_tensor("x", [BC, M, D], F32, kind="ExternalInput")
        yd = nc.dram_tensor("y", [BC, D], F32, kind="ExternalOutput")
        with tile.TileContext(nc) as tc:
            _emit(nc, tc, xd, yd)
        nc.compile()
        _NC_CACHE["nc"] = nc
    return _NC_CACHE["nc"]


def run_sharded(x, trace=False, **kwargs):
    nc = _get_nc()
    in_maps = [
        {"x": np.ascontiguousarray(x[k * BC:(k + 1) * BC])}
        for k in range(NCORES)
    ]
    res = run_bass_kernel_spmd(nc, in_maps, core_ids=list(range(NCORES)),
                               trace=trace, **kwargs)
    out = np.concatenate([res.results[k]["y"] for k in range(NCORES)], axis=0)
    return out, res


def kernel(**inputs):
    x = np.asarray(inputs["x"], dtype=np.float32)
    out, _ = run_sharded(x)
    return out


# revision 53
# speedup vs baseline: 1.9265x; 1.9265x over previous
"""BasicLS on 8 trn2 cores — fp16 stats + scaled solve, engine-balanced.

Pipeline per 4096-batch tile:
  1. DMA x tile [128, (c,m,d)] fp32.
  2. ACT cast+swizzle -> xh [128, (d, q, g, m)] fp16   (c = 4q+g)
  3. PE transposes of the 32 [128,(g,m)] blocks -> PSUM; dpair0 copied
     PSUM->SBUF on DVE, dpair1 via DMA -> Fall [128=(g,m), (d, q, p)] fp16.
  4. fp16 products: 6 cross on DVE, 1 square on DVE, 2 squares on ACT.
  5. PE matmuls with a sliding ones-pattern weight reduce over m into
     PSUM stats [52=(4s+g), 512]; fp32 accumulation.
  6. ACT copy PSUM->SBUF with 1/32 scale -> fp16 stats; PE transposes
     stat chunks back (fp16) -> ST2 [128=p, t, q, 52=(4s+g)] fp16.
  7. Solve on scaled stats (pivot 1): Schur-eliminate, symmetric 3x3
     adjugate; fp16 temporaries, fp32 det/recip chain. Half 0 runs
     entirely on GPSIMD overlapping the tiles 4..7 stats; half 1 is the
     only tail. Output DMA'd per half.
"""

import itertools
import os as _os

import numpy as np

import concourse.bacc as bacc
import concourse.tile as tile
from concourse import mybir
from concourse.bass_utils import run_bass_kernel_spmd
from concourse.masks import make_identity

F32 = mybir.dt.float32
F16 = mybir.dt.float16

B, M, D = 262144, 32, 4
NCORES = 8
BC = B // NCORES          # 32768
NT = 8
TB = BC // NT             # 4096
CPT = TB // 128           # 32 (c = 4q + g, q:8, g:4)
NQ, NG = 8, 4
W = NT * CPT              # 256

# stat order: 0..3 = T0..T3; 4 S01, 5 S02, 6 S03, 7 S11, 8 S12, 9 S13,
# 10 S22, 11 S23, 12 S33
CROSS = [(4, 0, 1), (5, 0, 2), (6, 0, 3), (8, 1, 2), (9, 1, 3), (11, 2, 3)]
SQ = [(7, 1), (10, 2), (12, 3)]
NS = 13


def _emit(nc, tc, xd, yd):
    V, G, A = nc.vector, nc.gpsimd, nc.scalar

    x_all = xd.ap().rearrange("(t p c) m d -> t p c m d", t=NT, p=128)
    y_all = yd.ap().rearrange("(t p c) d -> p t c d", t=NT, p=128)

    with (
        tc.tile_pool(name="const", bufs=1) as cpool,
        tc.tile_pool(name="xin", bufs=3) as xpool,
        tc.tile_pool(name="xh", bufs=3) as xhpool,
        tc.tile_pool(name="fall", bufs=3) as fpool,
        tc.tile_pool(name="pr", bufs=12) as prpool,
        tc.tile_pool(name="sst", bufs=4) as sspool,
        tc.tile_pool(name="stat", bufs=1) as spool,
        tc.tile_pool(name="solve", bufs=1) as lpool,
        tc.tile_pool(name="pp", bufs=6) as pppool,
        tc.tile_pool(name="acc", bufs=4) as apool,
        tc.tile_pool(name="pst", bufs=2, space="PSUM") as ptpool,
        tc.tile_pool(name="psp", bufs=3, space="PSUM") as sppool,
        tc.tile_pool(name="ps2", bufs=1, space="PSUM") as p2pool,
    ):
        ident16 = cpool.tile([128, 128], F16, name="ident16")
        make_identity(nc, ident16)
        # master ones-pattern weight: MW[32g+m, 48+g] = 1.
        # For stat s, lhsT = MW[:, 48-4s : 100-4s] places the group-g m-sum
        # of the rhs at output partition 4s+g.
        MW = cpool.tile([128, 100], F16, name="MW")
        V.memset(MW, 0.0)
        for g in range(NG):
            V.memset(MW[32 * g:32 * (g + 1), 48 + g:49 + g], 1.0)

        _skip_stats = _os.environ.get("KB_SKIP_STATS") == "1"
        _skip_solve = _os.environ.get("KB_SKIP_SOLVE") == "1"
        HT = NT // 2  # legacy name; halves are asymmetric below
        H0N, H1N = 4, 4  # tiles per solve half: big half overlaps on GPSIMD

        # per-batch stats (scaled by 1/32, fp16), split by t-half so the
        # first solve half's dependencies close after tile 3
        ST2h = [
            spool.tile([128, n, NQ, 52], F16, name=f"ST2_{hh}",
                       tag=f"ST2_{hh}")
            for hh, n in enumerate((H0N, H1N))
        ]
        if _skip_stats:
            V.memset(ST2h[0], 1.0)
            V.memset(ST2h[1], 1.0)

        OUT = lpool.tile([128, NT, CPT, D], F32, tag="OUT", name="OUT")
        OUT5 = OUT.rearrange("p t (q g) d -> p t q g d", q=NQ)

        def emit_solve(hh, tl=0, th=None, engines=None):
            """Generator: yields at chunk boundaries so emission can be
            interleaved with tile fronts (keeps engine streams from
            head-of-line blocking on solve deps)."""
            if th is None:
                th = H0N if hh == 0 else H1N

            def stat(s):
                return ST2h[hh][:, tl:th, :, 4 * s:4 * s + 4]

            a, b, c_, d = stat(7), stat(8), stat(9), stat(1)
            e, f_, g_ = stat(10), stat(11), stat(2)
            h, i_ = stat(12), stat(3)
            r0, r1, r2, r3 = stat(4), stat(5), stat(6), stat(0)

            # half 0 overlaps the tiles 4..7 stats work: pure GPSIMD so no
            # tile-engine stream is ever head-of-line blocked by solve deps.
            # half 1 is the tail: pure DVE — a single in-order stream has no
            # cross-engine semaphore hops on the critical path.
            sched = itertools.cycle(
                engines or ([G] if hh == 0 else [V, V, V, G]))
            SH = [128, th - tl, NQ, 4]

            def tmp(name, dt=F16, pool=None, tag=None):
                pool = pool or lpool
                name = f"{name}_h{hh}_{tl}"
                return pool.tile(SH, dt, tag=tag or name, name=name)

            qb = (0 if hh == 0 else H0N) + tl  # absolute tile base

            def emul(u, v, name, dt=F16, pool=None, tag=None):
                t_ = tmp(name, dt, pool, tag)
                next(sched).tensor_mul(out=t_, in0=u, in1=v)
                return t_

            def esub(u, v, name, dt=F16, pool=None, tag=None):
                t_ = tmp(name, dt, pool, tag)
                next(sched).tensor_sub(out=t_, in0=u, in1=v)
                return t_

            def eadd(u, v, name, dt=F16, pool=None, tag=None):
                t_ = tmp(name, dt, pool, tag)
                next(sched).tensor_add(out=t_, in0=u, in1=v)
                return t_

            pptag = f"pp{hh}_{tl}"

            def esq(u, name):  # u*u; ACT (idle in the tail) for half 1
                t_ = tmp(name)
                next(sched).tensor_mul(out=t_, in0=u, in1=u)
                return t_

            def m2(u, v, w, x_, name):  # u*v - w*x
                p1 = emul(u, v, name + "p1", F16, pppool, pptag)
                p2 = emul(w, x_, name + "p2", F16, pppool, pptag)
                return esub(p1, p2, name)

            # ---- Schur elimination of column 4 (pivot 1 after 1/32 scale) --
            dd = esq(d, "dd")
            gg = esq(g_, "gg")
            ii = esq(i_, "ii")
            dg = emul(d, g_, "dg", F16, pppool, pptag)
            di = emul(d, i_, "di", F16, pppool, pptag)
            gi = emul(g_, i_, "gi", F16, pppool, pptag)
            ap_ = esub(a, dd, "ap")
            bp = esub(b, dg, "bp")
            cp = esub(c_, di, "cp")
            ep = esub(e, gg, "ep")
            fp = esub(f_, gi, "fp")
            hp = esub(h, ii, "hp")
            p1_ = emul(r3, d, "c1p", F16, pppool, pptag)
            c1 = esub(r0, p1_, "c1")
            p2_ = emul(r3, g_, "c2p", F16, pppool, pptag)
            c2 = esub(r1, p2_, "c2")
            p3_ = emul(r3, i_, "c3p", F16, pppool, pptag)
            c3 = esub(r2, p3_, "c3")
            yield

            # ---- symmetric 3x3 adjugate solve ----
            fp2 = esq(fp, "fp2")
            cp2 = esq(cp, "cp2")
            bp2 = esq(bp, "bp2")
            eh = emul(ep, hp, "eh", F16, pppool, pptag)
            A11 = esub(eh, fp2, "A11")
            ah = emul(ap_, hp, "ah", F16, pppool, pptag)
            A22 = esub(ah, cp2, "A22")
            ae_ = emul(ap_, ep, "ae3", F16, pppool, pptag)
            A33 = esub(ae_, bp2, "A33")
            A12 = m2(cp, fp, bp, hp, "A12")
            A13 = m2(bp, fp, cp, ep, "A13")
            A23 = m2(cp, bp, ap_, fp, "A23")
            yield

            def dot3(u1, v1, u2, v2, u3, v3, name, mdt=F16):
                q1 = emul(u1, v1, name + "q1", mdt, pppool, pptag)
                q2 = emul(u2, v2, name + "q2", mdt, pppool, pptag)
                s_ = eadd(q1, q2, name + "s", mdt, apool, f"acc{hh}_{tl}")
                q3 = emul(u3, v3, name + "q3", mdt, pppool, pptag)
                return eadd(s_, q3, name, F32)

            det3 = dot3(ap_, A11, bp, A12, cp, A13, "det3")
            n1 = dot3(A11, c1, A12, c2, A13, c3, "n1")
            n2 = dot3(A12, c1, A22, c2, A23, c3, "n2")
            n3 = dot3(A13, c1, A23, c2, A33, c3, "n3")
            yield

            # z4 = (c4*det3 - (d n1 + g n2 + i n3)) * rdet
            dn = dot3(d, n1, g_, n2, i_, n3, "dn", F32)
            c4det = emul(r3, det3, "c4det", F32, pppool, pptag)
            diff = esub(c4det, dn, "diff", F32)
            yield

            # DVE recip emitted post-tile-7 for half 0: it waits on GPSIMD's
            # det3, and emitting it earlier head-of-line blocks tile work
            # queued behind it in the DVE stream.
            rdet = tmp("rdet", F32)
            scratch = tmp("rscratch", F32)
            V.reciprocal_approx_accurate(
                out=rdet.rearrange("p t q g -> p (t q g)"),
                in_=det3.rearrange("p t q g -> p (t q g)"),
                scratch=scratch.rearrange("p t q g -> p (t q g)"),
            )
            nrdet = tmp("nrdet", F32)
            V.tensor_scalar_mul(out=nrdet, in0=rdet, scalar1=-1.0)

            for comp, (dv, rv) in enumerate(
                [(n1, nrdet), (n2, nrdet), (n3, nrdet), (diff, rdet)]
            ):
                next(sched).tensor_mul(
                    out=OUT5[:, qb:qb + (th - tl), :, :, comp],
                    in0=dv, in1=rv,
                )

        def emit_tile(t):
            # Tiles 0..3: GPSIMD helps with tile work (it is otherwise idle).
            # Tiles 4..7: GPSIMD runs solve half 0, so its share moves to V/A.
            use_g = t < NT // 2
            xh = xhpool.tile([128, D, NQ, NG * M], F16, tag="xh", name="xh")
            if t == 0:
                # Split tile 0's DMA + cast by q-half so the first PE
                # transposes start after a half-tile DMA, not the full one.
                for h in range(4):
                    xth = cpool.tile([128, CPT // 4, M, D], F32,
                                     tag=f"xt0_{h}", name=f"xt0_{h}")
                    nc.sync.dma_start(
                        out=xth, in_=x_all[0][:, h * 8:(h + 1) * 8])
                    xswh = xth.rearrange("p (q g) m d -> p d q (g m)", q=2)
                    A.copy(out=xh[:, 0:2, 2 * h:2 * h + 2], in_=xswh[:, 0:2])
                    G.tensor_copy(
                        out=xh[:, 2:4, 2 * h:2 * h + 2], in_=xswh[:, 2:4])
            else:
                xt = xpool.tile([128, CPT, M, D], F32, tag="xt", name="xt")
                nc.sync.dma_start(out=xt, in_=x_all[t])
                # cast + swizzle (c,m,d) -> (d, q, g, m) in one ACT pass
                xsw = xt.rearrange("p (q g) m d -> p d q (g m)", q=NQ)
                A.copy(out=xh[:, 0:2], in_=xsw[:, 0:2])
                if use_g:
                    G.tensor_copy(out=xh[:, 2:4], in_=xsw[:, 2:4])
                else:
                    A.copy(out=xh[:, 2:4], in_=xsw[:, 2:4])
            # PE transposes: blocks (d, q): [128, (g,m)] -> [(g,m), 128]
            fall = fpool.tile([128, D, NQ, 128], F16, tag="fall", name="fall")
            for dpair in range(2):
                pt = ptpool.tile([128, 2, NQ, 128], F16, tag="pt", name="pt")
                for dd in range(2):
                    d = dpair * 2 + dd
                    for q in range(NQ):
                        nc.tensor.transpose(
                            pt[:, dd, q, :], xh[:, d, q, :], ident16
                        )
                if dpair == 0:
                    V.tensor_copy(out=fall[:, 0:2, :, :], in_=pt)
                elif use_g:
                    A.copy(out=fall[:, 2:4, :, :], in_=pt)
                else:
                    V.tensor_copy(out=fall[:, 2:4, :, :], in_=pt)
            # fp16 products (feature-major, unit stride)
            prods = {}
            for ci, (s, j, k) in enumerate(CROSS):
                pr = prpool.tile([128, NQ, 128], F16, tag="pr", name="pr")
                V.tensor_mul(out=pr, in0=fall[:, j], in1=fall[:, k])
                prods[s] = pr
            for sqi, (s, j) in enumerate(SQ):
                pr = prpool.tile([128, NQ, 128], F16, tag="pr", name="pr")
                if sqi == 1:
                    A.square(out=pr, in_=fall[:, j])
                else:
                    eng = V if (sqi == 0 or not use_g) else G
                    eng.tensor_mul(out=pr, in0=fall[:, j], in1=fall[:, j])
                prods[s] = pr
            # PE reduce over m: 13 accumulating matmuls per 512-col window
            SPT[t] = []
            for w in range(2):
                spt = sppool.tile([52, 512], F32, tag="spt", name="spt")
                for si, s in enumerate(range(NS)):
                    if s < 4:
                        rhs = fall[:, s, 4 * w:4 * w + 4, :]
                    else:
                        rhs = prods[s][:, 4 * w:4 * w + 4, :]
                    nc.tensor.matmul(
                        spt,
                        MW[:, 48 - 4 * s:100 - 4 * s],
                        rhs,
                        start=(si == 0),
                        stop=(si == NS - 1),
                    )
                SPT[t].append(spt)

        SPT, SST = {}, {}

        def emit_epi_a(t):
            # scale 1/32 (pivot -> 1) + cast fp16 during PSUM->SBUF.
            # Emitted AFTER tile t+1's front so ACT's in-order stream never
            # parks tile t+1's casts behind this PE-stats-dependent copy.
            SST[t] = []
            for w in range(2):
                sst = sspool.tile([52, 512], F16, tag="sst", name="sst")
                A.mul(out=sst, in_=SPT[t][w], mul=1.0 / 32.0)
                SST[t].append(sst)

        def emit_epi_b(t):
            # transpose stats back: chunks [52, 128] -> [128, 52] fp16
            pt2 = p2pool.tile([128, 2, 4, 52], F16, tag="pt2", name="pt2")
            for w in range(2):
                for cidx in range(4):
                    nc.tensor.transpose(
                        pt2[:, w, cidx, :],
                        SST[t][w][:, cidx * 128:(cidx + 1) * 128],
                        ident16[0:52, 0:52],
                    )
            for w in range(2):
                hh_ = 0 if t < H0N else 1
                dst = ST2h[hh_][:, t - hh_ * H0N, 4 * w:4 * w + 4, :]
                if w == 0:
                    V.tensor_copy(out=dst, in_=pt2[:, 0])
                else:
                    A.copy(out=dst, in_=pt2[:, 1])
            del SPT[t], SST[t]

        def emit_all_tiles(pump_cb=None):
            """Software-pipelined emission: front(t+1) goes out before tile
            t's stats epilogue, so no engine stream serializes a next-tile
            producer behind a this-tile consumer. Solve-half-0 chunks are
            interleaved once its stats (tiles 0..3) are complete."""
            if _skip_stats:
                return
            for t in range(NT):
                emit_tile(t)
                if t >= 1:
                    emit_epi_a(t - 1)
                    emit_epi_b(t - 1)
                if pump_cb is not None:
                    pump_cb(t)
            emit_epi_a(NT - 1)
            emit_epi_b(NT - 1)
            if pump_cb is not None:
                pump_cb(NT)

        if _skip_solve:
            emit_all_tiles()
            OUTs = lpool.tile([128, NT, CPT, D], F32, tag="OUT", name="OUTs")
            V.tensor_copy(out=OUTs, in_=ST2h[0].rearrange(
                "p t q s -> p (t q s)")[:, 0:NT * CPT * D].rearrange(
                "p (t c d) -> p t c d", t=NT, c=CPT))
            nc.sync.dma_start(out=y_all, in_=OUTs)
            return

        # Four quarter-solves: Q_k covers tiles (2k, 2k+1) and can start
        # as soon as its stats land (epi_b(2k+1)). Q0/Q1 run pure-GPSIMD
        # during the PE phase; only Q3 (plus Q2 spill) is tail. y DMAs ride
        # GPSIMD's SWDGE (no tile work queued behind them).
        YSL = [(0, H0N), (H0N, NT)]
        QS = [emit_solve(0, engines=[G]), None]
        START = [H0N, 99]

        def pump(t):
            for k, gen in enumerate(QS):
                if gen is not None and t >= START[k]:
                    if next(gen, "done") == "done":
                        QS[k] = None
                        lo, hi = YSL[k]
                        G.dma_start(out=y_all[:, lo:hi], in_=OUT[:, lo:hi])

        emit_all_tiles(pump_cb=pump)
        for k, gen in enumerate([QS[0], emit_solve(1, engines=[V, V, V, G])]):
            if gen is not None:
                for _ in gen:
                    pass
                lo, hi = YSL[k]
                G.dma_start(out=y_all[:, lo:hi], in_=OUT[:, lo:hi])


_NC_CACHE = {}


def _get_nc():
    if "nc" not in _NC_CACHE:
        nc = bacc.Bacc("TRN2", target_bir_lowering=False, debug=False,
                       num_devices=NCORES)
        xd = nc.dram_tensor("x", [BC, M, D], F32, kind="ExternalInput")
        yd = nc.dram_tensor("y", [BC, D], F32, kind="ExternalOutput")
        with tile.TileContext(nc) as tc:
            _emit(nc, tc, xd, yd)
        nc.compile()
        _NC_CACHE["nc"] = nc
    return _NC_CACHE["nc"]


def run_sharded(x, trace=False, **kwargs):
    nc = _get_nc()
    in_maps = [
        {"x": np.ascontiguousarray(x[k * BC:(k + 1) * BC])}
        for k in range(NCORES)
    ]
    res = run_bass_kernel_spmd(nc, in_maps, core_ids=list(range(NCORES)),
                               trace=trace, **kwargs)
    out = np.concatenate([res.results[k]["y"] for k in range(NCORES)], axis=0)
    return out, res


def kernel(**inputs):
    x = np.asarray(inputs["x"], dtype=np.float32)
    out, _ = run_sharded(x)
    return out


# revision 54
# speedup vs baseline: 1.9285x; 1.0011x over previous
"""BasicLS on 8 trn2 cores — fp16 stats + scaled solve, engine-balanced.

Pipeline per 4096-batch tile:
  1. DMA x tile [128, (c,m,d)] fp32.
  2. ACT cast+swizzle -> xh [128, (d, q, g, m)] fp16   (c = 4q+g)
  3. PE transposes of the 32 [128,(g,m)] blocks -> PSUM; dpair0 copied
     PSUM->SBUF on DVE, dpair1 via DMA -> Fall [128=(g,m), (d, q, p)] fp16.
  4. fp16 products: 6 cross on DVE, 1 square on DVE, 2 squares on ACT.
  5. PE matmuls with a sliding ones-pattern weight reduce over m into
     PSUM stats [52=(4s+g), 512]; fp32 accumulation.
  6. ACT copy PSUM->SBUF with 1/32 scale -> fp16 stats; PE transposes
     stat chunks back (fp16) -> ST2 [128=p, t, q, 52=(4s+g)] fp16.
  7. Solve on scaled stats (pivot 1): Schur-eliminate, symmetric 3x3
     adjugate; fp16 temporaries, fp32 det/recip chain. Half 0 runs
     entirely on GPSIMD overlapping the tiles 4..7 stats; half 1 is the
     only tail. Output DMA'd per half.
"""

import itertools
import os as _os

import numpy as np

import concourse.bacc as bacc
import concourse.tile as tile
from concourse import mybir
from concourse.bass_utils import run_bass_kernel_spmd
from concourse.masks import make_identity

F32 = mybir.dt.float32
F16 = mybir.dt.float16

B, M, D = 262144, 32, 4
NCORES = 8
BC = B // NCORES          # 32768
NT = 8
TB = BC // NT             # 4096
CPT = TB // 128           # 32 (c = 4q + g, q:8, g:4)
NQ, NG = 8, 4
W = NT * CPT              # 256

# stat order: 0..3 = T0..T3; 4 S01, 5 S02, 6 S03, 7 S11, 8 S12, 9 S13,
# 10 S22, 11 S23, 12 S33
CROSS = [(4, 0, 1), (5, 0, 2), (6, 0, 3), (8, 1, 2), (9, 1, 3), (11, 2, 3)]
SQ = [(7, 1), (10, 2), (12, 3)]
NS = 13


def _emit(nc, tc, xd, yd):
    V, G, A = nc.vector, nc.gpsimd, nc.scalar

    x_all = xd.ap().rearrange("(t p c) m d -> t p c m d", t=NT, p=128)
    y_all = yd.ap().rearrange("(t p c) d -> p t c d", t=NT, p=128)

    with (
        tc.tile_pool(name="const", bufs=1) as cpool,
        tc.tile_pool(name="xin", bufs=3) as xpool,
        tc.tile_pool(name="xh", bufs=4) as xhpool,
        tc.tile_pool(name="fall", bufs=4) as fpool,
        tc.tile_pool(name="pr", bufs=12) as prpool,
        tc.tile_pool(name="sst", bufs=4) as sspool,
        tc.tile_pool(name="stat", bufs=1) as spool,
        tc.tile_pool(name="solve", bufs=1) as lpool,
        tc.tile_pool(name="pp", bufs=6) as pppool,
        tc.tile_pool(name="acc", bufs=4) as apool,
        tc.tile_pool(name="pst", bufs=2, space="PSUM") as ptpool,
        tc.tile_pool(name="psp", bufs=3, space="PSUM") as sppool,
        tc.tile_pool(name="ps2", bufs=1, space="PSUM") as p2pool,
    ):
        ident16 = cpool.tile([128, 128], F16, name="ident16")
        make_identity(nc, ident16)
        # master ones-pattern weight: MW[32g+m, 48+g] = 1.
        # For stat s, lhsT = MW[:, 48-4s : 100-4s] places the group-g m-sum
        # of the rhs at output partition 4s+g.
        MW = cpool.tile([128, 100], F16, name="MW")
        V.memset(MW, 0.0)
        for g in range(NG):
            V.memset(MW[32 * g:32 * (g + 1), 48 + g:49 + g], 1.0)

        _skip_stats = _os.environ.get("KB_SKIP_STATS") == "1"
        _skip_solve = _os.environ.get("KB_SKIP_SOLVE") == "1"
        HT = NT // 2  # legacy name; halves are asymmetric below
        H0N, H1N = 4, 4  # tiles per solve half: big half overlaps on GPSIMD

        # per-batch stats (scaled by 1/32, fp16), split by t-half so the
        # first solve half's dependencies close after tile 3
        ST2h = [
            spool.tile([128, n, NQ, 52], F16, name=f"ST2_{hh}",
                       tag=f"ST2_{hh}")
            for hh, n in enumerate((H0N, H1N))
        ]
        if _skip_stats:
            V.memset(ST2h[0], 1.0)
            V.memset(ST2h[1], 1.0)

        OUT = lpool.tile([128, NT, CPT, D], F32, tag="OUT", name="OUT")
        OUT5 = OUT.rearrange("p t (q g) d -> p t q g d", q=NQ)

        def emit_solve(hh, tl=0, th=None, engines=None):
            """Generator: yields at chunk boundaries so emission can be
            interleaved with tile fronts (keeps engine streams from
            head-of-line blocking on solve deps)."""
            if th is None:
                th = H0N if hh == 0 else H1N

            def stat(s):
                return ST2h[hh][:, tl:th, :, 4 * s:4 * s + 4]

            a, b, c_, d = stat(7), stat(8), stat(9), stat(1)
            e, f_, g_ = stat(10), stat(11), stat(2)
            h, i_ = stat(12), stat(3)
            r0, r1, r2, r3 = stat(4), stat(5), stat(6), stat(0)

            # half 0 overlaps the tiles 4..7 stats work: pure GPSIMD so no
            # tile-engine stream is ever head-of-line blocked by solve deps.
            # half 1 is the tail: pure DVE — a single in-order stream has no
            # cross-engine semaphore hops on the critical path.
            sched = itertools.cycle(
                engines or ([G] if hh == 0 else [V, V, V, G]))
            SH = [128, th - tl, NQ, 4]

            def tmp(name, dt=F16, pool=None, tag=None):
                pool = pool or lpool
                name = f"{name}_h{hh}_{tl}"
                return pool.tile(SH, dt, tag=tag or name, name=name)

            qb = (0 if hh == 0 else H0N) + tl  # absolute tile base

            def emul(u, v, name, dt=F16, pool=None, tag=None):
                t_ = tmp(name, dt, pool, tag)
                next(sched).tensor_mul(out=t_, in0=u, in1=v)
                return t_

            def esub(u, v, name, dt=F16, pool=None, tag=None):
                t_ = tmp(name, dt, pool, tag)
                next(sched).tensor_sub(out=t_, in0=u, in1=v)
                return t_

            def eadd(u, v, name, dt=F16, pool=None, tag=None):
                t_ = tmp(name, dt, pool, tag)
                next(sched).tensor_add(out=t_, in0=u, in1=v)
                return t_

            pptag = f"pp{hh}_{tl}"

            def esq(u, name):  # u*u; ACT (idle in the tail) for half 1
                t_ = tmp(name)
                next(sched).tensor_mul(out=t_, in0=u, in1=u)
                return t_

            def m2(u, v, w, x_, name):  # u*v - w*x
                p1 = emul(u, v, name + "p1", F16, pppool, pptag)
                p2 = emul(w, x_, name + "p2", F16, pppool, pptag)
                return esub(p1, p2, name)

            # ---- Schur elimination of column 4 (pivot 1 after 1/32 scale) --
            dd = esq(d, "dd")
            gg = esq(g_, "gg")
            ii = esq(i_, "ii")
            dg = emul(d, g_, "dg", F16, pppool, pptag)
            di = emul(d, i_, "di", F16, pppool, pptag)
            gi = emul(g_, i_, "gi", F16, pppool, pptag)
            ap_ = esub(a, dd, "ap")
            bp = esub(b, dg, "bp")
            cp = esub(c_, di, "cp")
            ep = esub(e, gg, "ep")
            fp = esub(f_, gi, "fp")
            hp = esub(h, ii, "hp")
            p1_ = emul(r3, d, "c1p", F16, pppool, pptag)
            c1 = esub(r0, p1_, "c1")
            p2_ = emul(r3, g_, "c2p", F16, pppool, pptag)
            c2 = esub(r1, p2_, "c2")
            p3_ = emul(r3, i_, "c3p", F16, pppool, pptag)
            c3 = esub(r2, p3_, "c3")
            yield

            # ---- symmetric 3x3 adjugate solve ----
            fp2 = esq(fp, "fp2")
            cp2 = esq(cp, "cp2")
            bp2 = esq(bp, "bp2")
            eh = emul(ep, hp, "eh", F16, pppool, pptag)
            A11 = esub(eh, fp2, "A11")
            ah = emul(ap_, hp, "ah", F16, pppool, pptag)
            A22 = esub(ah, cp2, "A22")
            ae_ = emul(ap_, ep, "ae3", F16, pppool, pptag)
            A33 = esub(ae_, bp2, "A33")
            A12 = m2(cp, fp, bp, hp, "A12")
            A13 = m2(bp, fp, cp, ep, "A13")
            A23 = m2(cp, bp, ap_, fp, "A23")
            yield

            def dot3(u1, v1, u2, v2, u3, v3, name, mdt=F16):
                q1 = emul(u1, v1, name + "q1", mdt, pppool, pptag)
                q2 = emul(u2, v2, name + "q2", mdt, pppool, pptag)
                s_ = eadd(q1, q2, name + "s", mdt, apool, f"acc{hh}_{tl}")
                q3 = emul(u3, v3, name + "q3", mdt, pppool, pptag)
                return eadd(s_, q3, name, F32)

            det3 = dot3(ap_, A11, bp, A12, cp, A13, "det3")
            n1 = dot3(A11, c1, A12, c2, A13, c3, "n1")
            n2 = dot3(A12, c1, A22, c2, A23, c3, "n2")
            n3 = dot3(A13, c1, A23, c2, A33, c3, "n3")
            yield

            # z4 = (c4*det3 - (d n1 + g n2 + i n3)) * rdet
            dn = dot3(d, n1, g_, n2, i_, n3, "dn", F32)
            c4det = emul(r3, det3, "c4det", F32, pppool, pptag)
            diff = esub(c4det, dn, "diff", F32)
            yield

            # DVE recip emitted post-tile-7 for half 0: it waits on GPSIMD's
            # det3, and emitting it earlier head-of-line blocks tile work
            # queued behind it in the DVE stream.
            rdet = tmp("rdet", F32)
            scratch = tmp("rscratch", F32)
            V.reciprocal_approx_accurate(
                out=rdet.rearrange("p t q g -> p (t q g)"),
                in_=det3.rearrange("p t q g -> p (t q g)"),
                scratch=scratch.rearrange("p t q g -> p (t q g)"),
            )
            nrdet = tmp("nrdet", F32)
            V.tensor_scalar_mul(out=nrdet, in0=rdet, scalar1=-1.0)

            for comp, (dv, rv) in enumerate(
                [(n1, nrdet), (n2, nrdet), (n3, nrdet), (diff, rdet)]
            ):
                next(sched).tensor_mul(
                    out=OUT5[:, qb:qb + (th - tl), :, :, comp],
                    in0=dv, in1=rv,
                )

        def emit_tile(t):
            # Tiles 0..3: GPSIMD helps with tile work (it is otherwise idle).
            # Tiles 4..7: GPSIMD runs solve half 0, so its share moves to V/A.
            use_g = t < NT // 2
            xh = xhpool.tile([128, D, NQ, NG * M], F16, tag="xh", name="xh")
            if t == 0:
                # Split tile 0's DMA + cast by q-half so the first PE
                # transposes start after a half-tile DMA, not the full one.
                for h in range(4):
                    xth = cpool.tile([128, CPT // 4, M, D], F32,
                                     tag=f"xt0_{h}", name=f"xt0_{h}")
                    nc.sync.dma_start(
                        out=xth, in_=x_all[0][:, h * 8:(h + 1) * 8])
                    xswh = xth.rearrange("p (q g) m d -> p d q (g m)", q=2)
                    A.copy(out=xh[:, 0:2, 2 * h:2 * h + 2], in_=xswh[:, 0:2])
                    G.tensor_copy(
                        out=xh[:, 2:4, 2 * h:2 * h + 2], in_=xswh[:, 2:4])
            else:
                xt = xpool.tile([128, CPT, M, D], F32, tag="xt", name="xt")
                nc.sync.dma_start(out=xt, in_=x_all[t])
                # cast + swizzle (c,m,d) -> (d, q, g, m) in one ACT pass
                xsw = xt.rearrange("p (q g) m d -> p d q (g m)", q=NQ)
                A.copy(out=xh[:, 0:2], in_=xsw[:, 0:2])
                if use_g:
                    G.tensor_copy(out=xh[:, 2:4], in_=xsw[:, 2:4])
                else:
                    A.copy(out=xh[:, 2:4], in_=xsw[:, 2:4])
            # PE transposes: blocks (d, q): [128, (g,m)] -> [(g,m), 128]
            fall = fpool.tile([128, D, NQ, 128], F16, tag="fall", name="fall")
            for dpair in range(2):
                pt = ptpool.tile([128, 2, NQ, 128], F16, tag="pt", name="pt")
                for dd in range(2):
                    d = dpair * 2 + dd
                    for q in range(NQ):
                        nc.tensor.transpose(
                            pt[:, dd, q, :], xh[:, d, q, :], ident16
                        )
                if dpair == 0:
                    V.tensor_copy(out=fall[:, 0:2, :, :], in_=pt)
                elif use_g:
                    A.copy(out=fall[:, 2:4, :, :], in_=pt)
                else:
                    V.tensor_copy(out=fall[:, 2:4, :, :], in_=pt)
            # fp16 products (feature-major, unit stride)
            prods = {}
            for ci, (s, j, k) in enumerate(CROSS):
                pr = prpool.tile([128, NQ, 128], F16, tag="pr", name="pr")
                V.tensor_mul(out=pr, in0=fall[:, j], in1=fall[:, k])
                prods[s] = pr
            for sqi, (s, j) in enumerate(SQ):
                pr = prpool.tile([128, NQ, 128], F16, tag="pr", name="pr")
                if sqi == 1:
                    A.square(out=pr, in_=fall[:, j])
                else:
                    eng = V if (sqi == 0 or not use_g) else G
                    eng.tensor_mul(out=pr, in0=fall[:, j], in1=fall[:, j])
                prods[s] = pr
            # PE reduce over m: 13 accumulating matmuls per 512-col window
            SPT[t] = []
            for w in range(2):
                spt = sppool.tile([52, 512], F32, tag="spt", name="spt")
                for si, s in enumerate(range(NS)):
                    if s < 4:
                        rhs = fall[:, s, 4 * w:4 * w + 4, :]
                    else:
                        rhs = prods[s][:, 4 * w:4 * w + 4, :]
                    nc.tensor.matmul(
                        spt,
                        MW[:, 48 - 4 * s:100 - 4 * s],
                        rhs,
                        start=(si == 0),
                        stop=(si == NS - 1),
                    )
                SPT[t].append(spt)

        SPT, SST = {}, {}

        def emit_epi_a(t):
            # scale 1/32 (pivot -> 1) + cast fp16 during PSUM->SBUF.
            # Emitted AFTER tile t+1's front so ACT's in-order stream never
            # parks tile t+1's casts behind this PE-stats-dependent copy.
            SST[t] = []
            for w in range(2):
                sst = sspool.tile([52, 512], F16, tag="sst", name="sst")
                A.mul(out=sst, in_=SPT[t][w], mul=1.0 / 32.0)
                SST[t].append(sst)

        def emit_epi_b(t):
            # transpose stats back: chunks [52, 128] -> [128, 52] fp16
            pt2 = p2pool.tile([128, 2, 4, 52], F16, tag="pt2", name="pt2")
            for w in range(2):
                for cidx in range(4):
                    nc.tensor.transpose(
                        pt2[:, w, cidx, :],
                        SST[t][w][:, cidx * 128:(cidx + 1) * 128],
                        ident16[0:52, 0:52],
                    )
            for w in range(2):
                hh_ = 0 if t < H0N else 1
                dst = ST2h[hh_][:, t - hh_ * H0N, 4 * w:4 * w + 4, :]
                if w == 0:
                    V.tensor_copy(out=dst, in_=pt2[:, 0])
                else:
                    A.copy(out=dst, in_=pt2[:, 1])
            del SPT[t], SST[t]

        def emit_all_tiles(pump_cb=None):
            """Software-pipelined emission: front(t+1) goes out before tile
            t's stats epilogue, so no engine stream serializes a next-tile
            producer behind a this-tile consumer. Solve-half-0 chunks are
            interleaved once its stats (tiles 0..3) are complete."""
            if _skip_stats:
                return
            for t in range(NT):
                emit_tile(t)
                if t >= 1:
                    emit_epi_a(t - 1)
                    emit_epi_b(t - 1)
                if pump_cb is not None:
                    pump_cb(t)
            emit_epi_a(NT - 1)
            emit_epi_b(NT - 1)
            if pump_cb is not None:
                pump_cb(NT)

        if _skip_solve:
            emit_all_tiles()
            OUTs = lpool.tile([128, NT, CPT, D], F32, tag="OUT", name="OUTs")
            V.tensor_copy(out=OUTs, in_=ST2h[0].rearrange(
                "p t q s -> p (t q s)")[:, 0:NT * CPT * D].rearrange(
                "p (t c d) -> p t c d", t=NT, c=CPT))
            nc.sync.dma_start(out=y_all, in_=OUTs)
            return

        # Four quarter-solves: Q_k covers tiles (2k, 2k+1) and can start
        # as soon as its stats land (epi_b(2k+1)). Q0/Q1 run pure-GPSIMD
        # during the PE phase; only Q3 (plus Q2 spill) is tail. y DMAs ride
        # GPSIMD's SWDGE (no tile work queued behind them).
        YSL = [(0, H0N), (H0N, NT)]
        QS = [emit_solve(0, engines=[G]), None]
        START = [H0N, 99]

        def pump(t):
            for k, gen in enumerate(QS):
                if gen is not None and t >= START[k]:
                    if next(gen, "done") == "done":
                        QS[k] = None
                        lo, hi = YSL[k]
                        G.dma_start(out=y_all[:, lo:hi], in_=OUT[:, lo:hi])

        emit_all_tiles(pump_cb=pump)
        for k, gen in enumerate([QS[0], emit_solve(1, engines=[V, V, V, G])]):
            if gen is not None:
                for _ in gen:
                    pass
                lo, hi = YSL[k]
                G.dma_start(out=y_all[:, lo:hi], in_=OUT[:, lo:hi])


_NC_CACHE = {}


def _get_nc():
    if "nc" not in _NC_CACHE:
        nc = bacc.Bacc("TRN2", target_bir_lowering=False, debug=False,
                       num_devices=NCORES)
        xd = nc.dram_tensor("x", [BC, M, D], F32, kind="ExternalInput")
        yd = nc.dram_tensor("y", [BC, D], F32, kind="ExternalOutput")
        with tile.TileContext(nc) as tc:
            _emit(nc, tc, xd, yd)
        nc.compile()
        _NC_CACHE["nc"] = nc
    return _NC_CACHE["nc"]


def run_sharded(x, trace=False, **kwargs):
    nc = _get_nc()
    in_maps = [
        {"x": np.ascontiguousarray(x[k * BC:(k + 1) * BC])}
        for k in range(NCORES)
    ]
    res = run_bass_kernel_spmd(nc, in_maps, core_ids=list(range(NCORES)),
                               trace=trace, **kwargs)
    out = np.concatenate([res.results[k]["y"] for k in range(NCORES)], axis=0)
    return out, res


def kernel(**inputs):
    x = np.asarray(inputs["x"], dtype=np.float32)
    out, _ = run_sharded(x)
    return out
